# revision 2
# baseline (speedup 1.0000x reference)
"""Trainium2 Bass kernel for nn_BertSAE (top-k SAE encode/decode).

Data-parallel over the 8192-token dim across 8 NeuronCores (1024 tokens
per core, W_enc/W_dec replicated). Per core, three phases:

  1. Encode: pre = x @ W_enc^T + b_enc, computed exactly-to-fp32 via a
     bf16 hi/lo split (xh@Wh + xh@Wl + xl@Wh); fused per-chunk top-8
     stats (vector-engine max/max_index) while each [128, 2048] slice of
     pre is in SBUF; pre spilled to HBM.
  2. Top-32 per token from the 48x8 chunk-stat array: 4 rounds of
     max+match_replace for sorted values, index association via
     scalar_tensor_tensor(is_equal, mult, accum_out).
  3. Masked decode: sl = pre * (pre >= t32) as bf16 -> HBM, then
     recon = sl @ W_dec^T + b_dec with DMA-transposed sl tiles on PE.

Self-contained: hardcodes shapes from the problem spec.
"""
import numpy as np
import ml_dtypes

import bass_rust
import concourse.bass as bass
import concourse.tile as tile
import concourse.mybir as mybir
from concourse.bass_utils import run_bass_kernel_spmd

NCORES = 8
N, D, S, K = 8192, 768, 24576, 32
NT = N // NCORES            # tokens per core
TILES = NT // 128           # 8 token tiles per core
KC = D // 128               # 6 contraction chunks
SB = 2048                   # phase-1 latent block
NSB = S // SB               # 12
NB = SB // 512              # PSUM banks per block
CS = 512                    # top-k chunk size
C = S // CS                 # 48 chunks
CPB = SB // CS              # chunks per phase-1 block
SLB = 4096                  # phase-3a pre-stream block
NEG = -1e30
dt = mybir.dt


def _split_multi_waits(nc):
    # This container's walrus build rejects >1 sync-wait per instruction;
    # split extras onto wait-only NoOps on the same engine.
    for bbb in nc.bb_map.values():
        bb = bbb.bb
        out, changed = [], False
        for inst in bb.instructions:
            si = inst.sync_info
            waits = list(si.on_wait) if si is not None and si.on_wait else []
            if len(waits) > 1:
                changed = True
                for w in waits[:-1]:
                    nop = mybir.InstNoOp(
                        name=f"I-waitsplit-{nc.next_id()}", ins=[], outs=[])
                    nop.engine = inst.engine
                    nop.sync_info = bass_rust.SyncInfo(on_wait=[w], on_update=[])
                    nc.register_instruction(nop)
                    out.append(nop)
                si.on_wait = waits[-1:]
            out.append(inst)
        if changed:
            bb.instructions = out


def _build():
    nc = bass.Bass()
    xh_d = nc.declare_dram_parameter("xh", [D, NT], dt.bfloat16, isOutput=False)
    xl_d = nc.declare_dram_parameter("xl", [D, NT], dt.bfloat16, isOutput=False)
    wh_d = nc.declare_dram_parameter("wh", [D, S], dt.bfloat16, isOutput=False)
    wl_d = nc.declare_dram_parameter("wl", [D, S], dt.bfloat16, isOutput=False)
    wd_d = nc.declare_dram_parameter("wd", [S, D], dt.bfloat16, isOutput=False)
    benc_d = nc.declare_dram_parameter("bencb", [128, S], dt.float32, isOutput=False)
    bdec_d = nc.declare_dram_parameter("bdecb", [128, D], dt.float32, isOutput=False)
    recon_d = nc.declare_dram_parameter("recon", [NT, D], dt.float32, isOutput=True)
    tacts_d = nc.declare_dram_parameter("tacts", [NT, K], dt.float32, isOutput=True)
    tidx_d = nc.declare_dram_parameter("tidx", [NT, K], dt.int32, isOutput=True)

    pre_hbm = nc.dram_tensor("pre_hbm", [NT, S], dt.float32)
    sl_hbm = nc.dram_tensor("sl_hbm", [NT, S], dt.bfloat16)

    with tile.TileContext(nc) as tc:
        with tc.tile_pool(name="persist", bufs=1) as pp:
            xh_sb = pp.tile([128, KC, NT], dt.bfloat16, tag="xh")
            xl_sb = pp.tile([128, KC, NT], dt.bfloat16, tag="xl")
            for kc in range(KC):
                nc.sync.dma_start(xh_sb[:, kc, :], xh_d[kc * 128:(kc + 1) * 128, :])
                nc.sync.dma_start(xl_sb[:, kc, :], xl_d[kc * 128:(kc + 1) * 128, :])
            V = [pp.tile([128, C * 8], dt.float32, tag=f"V{t}", name=f"V{t}") for t in range(TILES)]
            I = [pp.tile([128, C * 8], dt.uint16, tag=f"I{t}", name=f"I{t}") for t in range(TILES)]
            vals = [pp.tile([128, K], dt.float32, tag=f"vals{t}", name=f"vals{t}") for t in range(TILES)]

            # ---------------- Phase 1: encode + chunk stats ----------------
            with tc.tile_pool(name="wpool", bufs=2) as wp, \
                 tc.tile_pool(name="bpool", bufs=2) as bp, \
                 tc.tile_pool(name="prepool", bufs=3) as prp, \
                 tc.tile_pool(name="p1psum", bufs=2, space="PSUM") as p1ps:
                for sb in range(NSB):
                    wh_t = wp.tile([128, KC, SB], dt.bfloat16, tag="wh")
                    wl_t = wp.tile([128, KC, SB], dt.bfloat16, tag="wl")
                    for kc in range(KC):
                        nc.sync.dma_start(
                            wh_t[:, kc, :],
                            wh_d[kc * 128:(kc + 1) * 128, sb * SB:(sb + 1) * SB])
                        nc.sync.dma_start(
                            wl_t[:, kc, :],
                            wl_d[kc * 128:(kc + 1) * 128, sb * SB:(sb + 1) * SB])
                    bias_t = bp.tile([128, SB], dt.float32, tag="bias")
                    nc.sync.dma_start(bias_t[:], benc_d[:, sb * SB:(sb + 1) * SB])
                    for t in range(TILES):
                        ps = p1ps.tile([128, SB], dt.float32, tag="ps")
                        terms = ((xh_sb, wh_t), (xh_sb, wl_t), (xl_sb, wh_t))
                        for ti, (xs, ws) in enumerate(terms):
                            for kc in range(KC):
                                for b in range(NB):
                                    nc.tensor.matmul(
                                        ps[:, b * 512:(b + 1) * 512],
                                        xs[:, kc, t * 128:(t + 1) * 128],
                                        ws[:, kc, b * 512:(b + 1) * 512],
                                        start=(ti == 0 and kc == 0),
                                        stop=(ti == 2 and kc == KC - 1))
                        pre_t = prp.tile([128, SB], dt.float32, tag="pre")
                        nc.vector.tensor_add(pre_t[:], ps[:], bias_t[:])
                        for j in range(CPB):
                            cg = sb * CPB + j
                            nc.vector.max(V[t][:, cg * 8:(cg + 1) * 8],
                                          pre_t[:, j * CS:(j + 1) * CS])
                            nc.vector.max_index(I[t][:, cg * 8:(cg + 1) * 8],
                                                V[t][:, cg * 8:(cg + 1) * 8],
                                                pre_t[:, j * CS:(j + 1) * CS])
                        nc.sync.dma_start(
                            pre_hbm[t * 128:(t + 1) * 128, sb * SB:(sb + 1) * SB],
                            pre_t[:])

            # ---------------- Phase 2: top-32 values + indices ----------------
            with tc.tile_pool(name="p2", bufs=1) as p2:
                base = p2.tile([128, C, 8], dt.int32, tag="base")
                nc.gpsimd.iota(base[:], [[CS, C], [0, 8]], channel_multiplier=0)
                for t in range(TILES):
                    i32 = p2.tile([128, C * 8], dt.int32, tag="i32")
                    nc.vector.tensor_copy(i32[:], I[t][:])
                    gi = p2.tile([128, C * 8], dt.int32, tag="gi")
                    nc.vector.tensor_add(gi[:], i32[:],
                                         base[:].rearrange("p a b -> p (a b)"))
                    G = p2.tile([128, C * 8], dt.float32, tag="G")
                    nc.vector.tensor_copy(G[:], gi[:])

                    va = p2.tile([128, C * 8], dt.float32, tag="va")
                    vb = p2.tile([128, C * 8], dt.float32, tag="vb")
                    nc.vector.tensor_copy(va[:], V[t][:])
                    cur, nxt = va, vb
                    for r in range(K // 8):
                        nc.vector.max(vals[t][:, r * 8:(r + 1) * 8], cur[:])
                        nc.vector.match_replace(nxt[:], vals[t][:, r * 8:(r + 1) * 8],
                                                cur[:], NEG)
                        cur, nxt = nxt, cur

                    jf = p2.tile([128, K], dt.float32, tag="jf")
                    scr = p2.tile([128, C * 8], dt.float32, tag="scr")
                    for k in range(K):
                        nc.vector.scalar_tensor_tensor(
                            scr[:], V[t][:], vals[t][:, k:k + 1], G[:],
                            mybir.AluOpType.is_equal, mybir.AluOpType.mult,
                            accum_out=jf[:, k:k + 1])
                    ji = p2.tile([128, K], dt.int32, tag="ji")
                    nc.vector.tensor_copy(ji[:], jf[:])
                    nc.sync.dma_start(tacts_d[t * 128:(t + 1) * 128, :], vals[t][:])
                    nc.sync.dma_start(tidx_d[t * 128:(t + 1) * 128, :], ji[:])

            # ---------------- Phase 3a: sl = pre * (pre >= t32) ----------------
            with tc.tile_pool(name="p3a", bufs=2) as p3a:
                for t in range(TILES):
                    for q in range(S // SLB):
                        prt = p3a.tile([128, SLB], dt.float32, tag="prt")
                        nc.sync.dma_start(
                            prt[:],
                            pre_hbm[t * 128:(t + 1) * 128, q * SLB:(q + 1) * SLB])
                        slt = p3a.tile([128, SLB], dt.bfloat16, tag="slt")
                        nc.vector.scalar_tensor_tensor(
                            slt[:], prt[:], vals[t][:, K - 1:K], prt[:],
                            mybir.AluOpType.is_ge, mybir.AluOpType.mult)
                        nc.sync.dma_start(
                            sl_hbm[t * 128:(t + 1) * 128, q * SLB:(q + 1) * SLB],
                            slt[:])

            # ---------------- Phase 3b: recon = sl @ W_dec^T + b_dec ----------
            with tc.tile_pool(name="p3b", bufs=3) as p3b, \
                 tc.tile_pool(name="outp", bufs=2) as op, \
                 tc.tile_pool(name="p3psum", bufs=1, space="PSUM") as p3ps:
                bdec_t = pp.tile([128, D], dt.float32, tag="bdec")
                nc.sync.dma_start(bdec_t[:], bdec_d[:])
                NSC = S // 128   # 192
                for g in range(2):
                    rps = [p3ps.tile([128, D], dt.float32, tag=f"rps{u}", name=f"rps{g}_{u}")
                           for u in range(4)]
                    for sc in range(NSC):
                        slT = p3b.tile([128, 512], dt.bfloat16, tag="slT")
                        nc.sync.dma_start_transpose(
                            slT[:],
                            sl_hbm[g * 512:(g + 1) * 512, sc * 128:(sc + 1) * 128])
                        wdt = p3b.tile([128, D], dt.bfloat16, tag="wdt")
                        nc.sync.dma_start(wdt[:], wd_d[sc * 128:(sc + 1) * 128, :])
                        for u in range(4):
                            nc.tensor.matmul(
                                rps[u][:, 0:512], slT[:, u * 128:(u + 1) * 128],
                                wdt[:, 0:512],
                                start=(sc == 0), stop=(sc == NSC - 1))
                            nc.tensor.matmul(
                                rps[u][:, 512:768], slT[:, u * 128:(u + 1) * 128],
                                wdt[:, 512:768],
                                start=(sc == 0), stop=(sc == NSC - 1))
                    for u in range(4):
                        rsb = op.tile([128, D], dt.float32, tag="rsb")
                        nc.vector.tensor_add(rsb[:], rps[u][:], bdec_t[:])
                        row = (g * 4 + u) * 128
                        nc.sync.dma_start(recon_d[row:row + 128, :], rsb[:])

    _split_multi_waits(nc)
    return nc


_NC = None


def _get_nc():
    global _NC
    if _NC is None:
        _NC = _build()
    return _NC


def kernel(x, W_enc, b_enc, W_dec, b_dec):
    bf16 = ml_dtypes.bfloat16
    f32 = np.float32
    x = np.asarray(x, dtype=f32)
    W_enc = np.asarray(W_enc, dtype=f32)
    b_enc = np.asarray(b_enc, dtype=f32)
    W_dec = np.asarray(W_dec, dtype=f32)
    b_dec = np.asarray(b_dec, dtype=f32)

    xT = np.ascontiguousarray(x.T)                       # [D, N]
    xh = xT.astype(bf16)
    xl = (xT - xh.astype(f32)).astype(bf16)
    WT = np.ascontiguousarray(W_enc.T)                   # [D, S]
    wh = WT.astype(bf16)
    wl = (WT - wh.astype(f32)).astype(bf16)
    wd = np.ascontiguousarray(W_dec.T).astype(bf16)      # [S, D]
    bencb = np.ascontiguousarray(np.broadcast_to(b_enc, (128, S)))
    bdecb = np.ascontiguousarray(np.broadcast_to(b_dec, (128, D)))

    in_maps = []
    for c in range(NCORES):
        sl_ = slice(c * NT, (c + 1) * NT)
        in_maps.append({
            "xh": np.ascontiguousarray(xh[:, sl_]),
            "xl": np.ascontiguousarray(xl[:, sl_]),
            "wh": wh, "wl": wl, "wd": wd,
            "bencb": bencb, "bdecb": bdecb,
        })

    nc = _get_nc()
    res = run_bass_kernel_spmd(nc, in_maps, list(range(NCORES)))
    recon = np.concatenate([res.results[c]["recon"] for c in range(NCORES)], axis=0)
    tacts = np.concatenate([res.results[c]["tacts"] for c in range(NCORES)], axis=0)
    tidx = np.concatenate([res.results[c]["tidx"] for c in range(NCORES)], axis=0)
    return recon, tacts, tidx
